# revision 14
# baseline (speedup 1.0000x reference)
"""Trainium2 Bass kernel for nn_DataEmbedding (embedding_lookup).

Reference computation (B=32, L=4096, C_IN=7, D=512):
  out = value_emb + pos_emb + temp_emb
  value_emb = TokenEmbedding(x) @ proj_w.T + proj_b   (73+1 tiny conv1d's, k=8)
  pos_emb   = sinusoid_table(L, D)
  temp_emb  = sum of 4 fixed sinusoid-table lookups from x_mark (indices in [0,7))

Device algorithm (per core, 4 batches):
  * TokenEmbedding+projection collapse into ONE size-8 conv over L:
      value_emb[b,l,d] = sum_{m,c} A[d,m,c] * xpad[b, l+m, c] + proj_b[d]
    with A = einsum(proj_w[:, :511].reshape(D,73,7), kernels[:73]) + c==0 term.
  * The 4 temporal lookups are a 28-row one-hot matmul (tables only ever
    indexed at rows 0..6 where all four sinusoid tables agree).
  * The whole stationary operand (one-hot rows stacked over im2col rows)
    is built host-side — a pure relayout of x / x_mark — so each batch
    needs exactly ONE input DMA.  One K=84 bf16 matmul per 128-position
    tile does all the math; pos_emb + proj_b are one [L, D] bf16 table
    added during PSUM eviction.

Performance structure (PE-bound: LDW+MM pairs measure 677ns sustained
on this part, 128 pairs ~= 87us/core floor; everything else sits under):
  * all-bf16 (tolerance 2e-2, actual err ~6e-3).
  * positions interleaved stride-8 within each 1024-position group (the
    interleave is applied host-side so device APs are contiguous):
    PSUM partition p always holds 8 consecutive output rows -> every
    output DMA is one fully contiguous 1MB transfer (128 x 8KB), pos
    loads likewise.
  * PSUM eviction at 2-bank granularity (FD=1024) to amortize per-op
    overhead and halve semaphore traffic: per 4 pairs, 2 go DVE-direct
    (tensor_tensor add from PSUM), 2 go ScE-copy + DVE bf16 add (2x).
  * output DMAs dispatch on the Act HWDGE ring, inputs on SP, so input
    dispatches never queue behind 1MB output transfers.

Sharding: pure data parallel over batch: 32 batches -> 8 cores x 4 batches.
"""

import os
import sys
import ml_dtypes
import numpy as np

for _p in ("/opt/trn_rl_repo", "/opt/pypackages"):
    if os.path.isdir(_p) and _p not in sys.path:
        sys.path.append(_p)

from contextlib import ExitStack

import concourse.bass as bass
import concourse.tile as tile
from concourse import bacc, mybir
from concourse.bass_utils import run_bass_kernel_spmd

# ---------------------------------------------------------------- constants
B, L, C_IN, D = 32, 4096, 7, 512
KS, NK, M = 8, 74, 7          # kernel_size, num_kernels, history
PROJ_IN = 73 * C_IN + 1       # 512
N_CORES = 8
NB = B // N_CORES             # batches per core = 4
KIM = KS * C_IN               # im2col rows = 56
KOH = 4 * 7                   # one-hot rows = 28
KTOT = KIM + KOH              # fused contraction = 84
P = 128                       # positions per matmul tile
GT = 8                        # tiles per group (position stride)
G = P * GT                    # positions per group = 1024
NG = L // G                   # groups per batch = 4
PAIR = 2 * D                  # eviction unit: 2 PSUM banks = 1024

F32 = mybir.dt.float32
BF16 = mybir.dt.bfloat16


def _sinusoid_table(n, d):
    pos = np.arange(n, dtype=np.float32)[:, None]
    div = np.exp(np.arange(0, d, 2, dtype=np.float32) * (-np.log(10000.0) / d))
    tab = np.zeros((n, d), dtype=np.float32)
    tab[:, 0::2] = np.sin(pos * div)
    tab[:, 1::2] = np.cos(pos * div)
    return tab


_POS_CACHE = None


def _pos_const():
    global _POS_CACHE
    if _POS_CACHE is None:
        _POS_CACHE = _sinusoid_table(L, D)
    return _POS_CACHE


# column permutation: device matmul tile (g, t) takes columns
# [ (g*GT + t)*P : +P ] of the permuted operand; column (g,t,p) must hold
# position g*G + 8p + t so that PSUM partition p = output row g*G+8p+t.
_PERM = None


def _col_perm():
    global _PERM
    if _PERM is None:
        l = np.arange(L)
        g, r = l // G, l % G
        t, p = r // P, r % P
        _PERM = (g * G + p * GT + t).astype(np.int64)  # perm[j'] = source pos
    return _PERM


def _host_prep(x, x_mark, kernels, proj_w, proj_b):
    """Build per-core inputs. All heavy math stays on device; this is layout
    glue plus the tiny [512,511]x[73,8] weight fold."""
    x = np.asarray(x, dtype=np.float32)
    x_mark = np.asarray(x_mark)
    kernels = np.asarray(kernels, dtype=np.float32)
    proj_w = np.asarray(proj_w, dtype=np.float32)
    proj_b = np.asarray(proj_b, dtype=np.float32)

    # full stationary operand [B, 84, L] bf16 (pure relayout of x/x_mark):
    #   rows 0..27   one-hot: row 7j+v = (x_mark[:, :, j] == v)
    #   rows 28..83  im2col:  row 28+7m+c = xpad[:, c, l+m]
    lhs = np.empty((B, KTOT, L), dtype=ml_dtypes.bfloat16)
    xm = x_mark.astype(np.int64)
    oh = xm[:, :, :, None] == np.arange(7)[None, None, None, :]   # [B,L,4,7]
    lhs[:, :KOH, :] = oh.transpose(0, 2, 3, 1).reshape(B, KOH, L)
    xpad = np.zeros((B, C_IN, L + KS), dtype=np.float32)
    xpad[:, :, M : M + L] = x.transpose(0, 2, 1)
    for m in range(KS):
        lhs[:, KOH + C_IN * m : KOH + C_IN * (m + 1), :] = xpad[
            :, :, m : m + L
        ]
    # stride-8 position interleave (see _col_perm)
    lhs = np.ascontiguousarray(lhs[:, :, _col_perm()])

    # fused conv weight A[d, m, c]
    p3 = proj_w[:, : 73 * C_IN].reshape(D, 73, C_IN)
    A = np.einsum("dkc,km->dmc", p3, kernels[:73], dtype=np.float32)
    A[:, :, 0] += np.outer(proj_w[:, 511], kernels[73])
    w_pack = A.transpose(1, 2, 0).reshape(KIM, D)  # row 7m+c

    # temporal tables: all four sinusoid tables agree on rows 0..6.
    tab7 = _sinusoid_table(7, D)  # [7, D]
    wtab = np.concatenate([np.tile(tab7, (4, 1)), w_pack], axis=0)  # [84, D]
    wtab = np.ascontiguousarray(wtab.astype(ml_dtypes.bfloat16))

    # positional + bias table (bf16: |values| <= ~1, rounding ~2e-3 abs,
    # negligible vs output scale ~22), rows in interleaved order so the
    # SBUF tile [128, NG*GT*D] has partition p = rows {g*G+8p+t}.
    posb = (_pos_const() + proj_b[None, :]).astype(ml_dtypes.bfloat16)
    # row r of interleaved table = position g*G + 8p + t where the SBUF
    # flat index is ((p * NG) + g) * GT + t ... simpler: build per-partition
    # layout directly: part p, free [g, t, d] = posb[g*G + 8p + t, d]
    pos_il = posb.reshape(NG, P, GT, D)            # [g, p, t, d]
    pos_il = np.ascontiguousarray(
        pos_il.transpose(1, 0, 2, 3).reshape(P, NG * GT * D)
    )  # [p, (g t d)]

    in_maps = []
    for core in range(N_CORES):
        sl = slice(core * NB, (core + 1) * NB)
        in_maps.append(
            {
                "lhs": np.ascontiguousarray(lhs[sl]),
                "wtab": wtab,
                "posil": pos_il,
            }
        )
    return in_maps


# ---------------------------------------------------------------- bass build
def build_nc(stage_bufs=6):
    nc = bacc.Bacc("TRN2", target_bir_lowering=False, debug=False)

    lhs_d = nc.dram_tensor("lhs", (NB, KTOT, L), BF16, kind="ExternalInput")
    wtab_d = nc.dram_tensor("wtab", (KTOT, D), BF16, kind="ExternalInput")
    posil_d = nc.dram_tensor("posil", (P, NG * GT * D), BF16,
                             kind="ExternalInput")
    out_d = nc.dram_tensor("out", (NB, L, D), BF16, kind="ExternalOutput")

    with tile.TileContext(nc) as tc, ExitStack() as ctx:
        dma = nc.sync        # input DMAs: SP HWDGE ring
        odma = nc.scalar     # output DMAs: Act HWDGE ring
        consts = ctx.enter_context(tc.tile_pool(name="consts", bufs=1))
        lhs_pool = ctx.enter_context(tc.tile_pool(name="lhsp", bufs=2))
        stage_pool = ctx.enter_context(tc.tile_pool(name="stage", bufs=stage_bufs))
        psum_pool = ctx.enter_context(
            tc.tile_pool(name="psum", bufs=4, space="PSUM")
        )

        # Startup order is the whole ramp-up story: the group-0 pos chunk
        # gates the first PSUM evictions (which gate the PE once all 8
        # banks fill), lhs0's group-0 chunk + wtab gate the first matmul.
        # Everything else follows behind on the ring.
        pos_s = consts.tile([P, NG * GT * D], BF16, tag="pos")
        lhs0 = lhs_pool.tile([KTOT, L], BF16, tag="lhs", name="lhs0")
        GTD = GT * D
        dma.dma_start(lhs0[:, 0:G], lhs_d.ap()[0, :, 0:G])
        wtab_s = consts.tile([KTOT, D], BF16, tag="wtab")
        dma.dma_start(wtab_s[:], wtab_d.ap())
        dma.dma_start(pos_s[:, 0:GTD], posil_d.ap()[:, 0:GTD])
        for g in range(1, NG):
            dma.dma_start(
                lhs0[:, g * G : (g + 1) * G], lhs_d.ap()[0, :, g * G : (g + 1) * G]
            )
            dma.dma_start(
                pos_s[:, g * GTD : (g + 1) * GTD],
                posil_d.ap()[:, g * GTD : (g + 1) * GTD],
            )

        lhs = lhs0
        for b in range(NB):
            if b + 1 < NB:
                next_lhs = lhs_pool.tile([KTOT, L], BF16, tag="lhs",
                                         name=f"lhs{b + 1}")
                dma.dma_start(next_lhs[:], lhs_d.ap()[b + 1])
            else:
                next_lhs = None
            for g in range(NG):
                stage = stage_pool.tile([P, GT * D], BF16, tag="stage")
                dst3 = out_d.ap()[b, g * G : (g + 1) * G, :]
                dst3 = dst3.rearrange("(p t) d -> p t d", p=P)
                for j in range(4):          # 4 pair-units of 2 tiles
                    ps = psum_pool.tile([P, PAIR], F32, tag="ps")
                    for h in range(2):
                        t = 2 * j + h
                        nc.tensor.matmul(
                            ps[:, D * h : D * (h + 1)],
                            lhs[:, (g * GT + t) * P : (g * GT + t + 1) * P],
                            wtab_s[:],
                            start=True,
                            stop=True,
                        )
                    ssl = slice(PAIR * j, PAIR * (j + 1))
                    psl = slice((g * GT + 2 * j) * D, (g * GT + 2 * j + 2) * D)
                    if j % 2 == 0:
                        # DVE: add pos straight out of PSUM (1x, FD=1024)
                        nc.vector.tensor_tensor(
                            out=stage[:, ssl],
                            in0=ps[:],
                            in1=pos_s[:, psl],
                            op=mybir.AluOpType.add,
                        )
                    else:
                        # ScE copy PSUM->SBUF, then DVE bf16 add (2x)
                        nc.scalar.copy(stage[:, ssl], ps[:])
                        nc.vector.tensor_tensor(
                            out=stage[:, ssl],
                            in0=stage[:, ssl],
                            in1=pos_s[:, psl],
                            op=mybir.AluOpType.add,
                        )
                    if j % 2 == 1:
                        # dispatch the finished half (t = 2j-2 .. 2j+1) right
                        # away: partition p covers rows g*G+8p+t, 4KB
                        # contiguous per partition.  Alternating rings.
                        half = j // 2
                        hdst = dst3[:, 4 * half : 4 * half + 4, :]
                        hsrc = stage[:, 2048 * half : 2048 * (half + 1)]
                        eng = odma if (g * 2 + half) % 2 == 0 else dma
                        eng.dma_start(
                            hdst, hsrc.rearrange("p (t d) -> p t d", d=D)
                        )
            lhs = next_lhs

    nc.compile()
    return nc


_NC_CACHE = None


def _get_nc():
    global _NC_CACHE
    if _NC_CACHE is None:
        _NC_CACHE = build_nc()
    return _NC_CACHE


TRACE = False          # set by test.py to capture an NTFF profile
LAST_RESULT = None     # BassKernelResults of the most recent run


def _run_once(in_maps):
    global LAST_RESULT
    nc = _get_nc()
    res = run_bass_kernel_spmd(
        nc, in_maps, core_ids=list(range(N_CORES)), trace=TRACE
    )
    LAST_RESULT = res
    return np.concatenate(
        [np.asarray(r["out"], dtype=np.float32) for r in res.results], axis=0
    )


def _run_subprocess(inputs):
    """Crash-isolated fallback: run in a fresh interpreter (a device fault can
    wedge the parent process's jax runtime)."""
    import pickle
    import subprocess
    import tempfile

    with tempfile.TemporaryDirectory() as td:
        fin = os.path.join(td, "in.pkl")
        fout = os.path.join(td, "out.npy")
        with open(fin, "wb") as f:
            pickle.dump(inputs, f)
        code = (
            "import pickle, numpy as np, sys;"
            f"sys.path.insert(0, {os.path.dirname(os.path.abspath(__file__))!r});"
            "import kernel as K;"
            f"ins = pickle.load(open({fin!r}, 'rb'));"
            "out = K._run_once(K._host_prep(**ins));"
            f"np.save({fout!r}, out)"
        )
        subprocess.run([sys.executable, "-c", code], check=True, timeout=1800)
        return np.load(fout)


def kernel(x, x_mark, kernels, proj_w, proj_b):
    inputs = dict(x=x, x_mark=x_mark, kernels=kernels, proj_w=proj_w,
                  proj_b=proj_b)
    in_maps = _host_prep(**inputs)
    # the TRN fleet shows rare transient NRT_EXEC_UNIT_UNRECOVERABLE faults;
    # retry in-process first, then in fresh subprocesses.
    for attempt in range(2):
        try:
            return _run_once(in_maps)
        except Exception:
            pass
    for attempt in range(3):
        try:
            return _run_subprocess(inputs)
        except Exception:
            if attempt == 2:
                raise
    raise RuntimeError("unreachable")
